# revision 38
# baseline (speedup 1.0000x reference)
"""KMeans min-distance loss kernel for Trainium2 (8 NeuronCores, SPMD).

Problem: features [262144, 128] f32, centers [256, 128] f32.
  d2[n,k] = ||f_n||^2 + ||c_k||^2 - 2 f_n.c_k ; out = mean_n sqrt(min_k d2)

Sharding: data-parallel over N (32768 rows per core), centers replicated.
Each core returns [128] partial sums of min-distances; host reduces.

Per-core pipeline (fp8 DoubleRow matmul carries cross + f2 + c2):
  - SWDGE cast-DMA staged groups (0.5-2MB): f32 dram -> bf16 sbuf
  - PE transpose (bf16) chunks -> PSUM, batches of 8 per PSUM bank
  - ACT evacuates PSUM -> fT fp8: copy (features) + Square (squares)
  - one fp8 DoubleRow matmul per chunk, contraction 256:
      rows   0-127: f[n,d] x -2c[k,d]
      rows 128-253: f[n,d]^2 (d<126) x 1.0          (-> ||f||^2 partial)
      rows 254-255: 1.0 x c2a[k], 1.0 x c2b[k]      (-> centered ||c||^2,
                                                     fp8 error-feedback pair)
  - DVE segmented tensor_reduce min over k -> m[:, 4]  (the pacing
    engine: 64 x ~1.2us, PSUM f32 streams at 1 elem/cycle)
  - dims 126/127 squares on GpSimd from the [n,d] tile (their ktile-1
    rows carry the c2 constants instead)
  - tail in two parts (split overlaps the last group): Pool adds, ACT
    sqrt(d2 + mean_c2) accum -> out [128, 2] partial sums; host reduces
"""

import sys

for p in ("/opt/trn_rl_repo", "/opt/trn_rl_repo/concourse"):
    if p not in sys.path:
        sys.path.insert(0, p)

import numpy as np

N_TOTAL = 262144
K = 256
D = 128
N_CORES = 8
N_PER_CORE = N_TOTAL // N_CORES  # 32768
P = 128
CHUNKS = N_PER_CORE // P         # 256 chunks of 128 rows
TB = 8                           # chunks per transpose/evac batch
TG = 4                           # chunks per cross/reduce batch
FTBUFS = 4                       # fT buffer rotation depth
# staged DMA group sizes (chunks): small first groups so compute starts
# while the bulk still streams; 2MB groups at steady state; small final
# groups so the epilogue tail has little left to wait for
GROUP_SIZES = [8, 8, 16] + [32] * 6 + [16] * 2
HWDGE_GROUPS = 2                 # first groups load as raw f32 via the
                                 # (earlier-ready) Sync queue + ACT cast
TAIL_SPLIT = 240                 # tail part A covers chunks [0, 240)

_compiled = None


def _build():
    import concourse.bass as bass
    import concourse.bacc as bacc
    import concourse.tile as tile
    from concourse import mybir

    f32 = mybir.dt.float32
    bf16 = mybir.dt.bfloat16
    fp8 = mybir.dt.float8e4
    Alu = mybir.AluOpType
    Act = mybir.ActivationFunctionType

    nc = bacc.Bacc(
        "TRN2", target_bir_lowering=False, debug=False, num_devices=N_CORES
    )

    feats = nc.dram_tensor("features", [N_PER_CORE, D], f32, kind="ExternalInput").ap()
    ctp = nc.dram_tensor("ctp", [P, 2 * K], fp8, kind="ExternalInput").ap()
    cones = nc.dram_tensor("cones", [2, TB * P], fp8, kind="ExternalInput").ap()
    ident = nc.dram_tensor("ident", [P, P], bf16, kind="ExternalInput").ap()
    c2mean = nc.dram_tensor("c2mean", [P, 1], f32, kind="ExternalInput").ap()
    out = nc.dram_tensor("out", [P, 2], f32, kind="ExternalOutput").ap()

    with tile.TileContext(nc) as tc:
        with (
            tc.tile_pool(name="consts", bufs=1) as consts,
            tc.tile_pool(name="featg", bufs=4) as featg_pool,
            tc.tile_pool(name="coll", bufs=1) as coll,
            tc.tile_pool(name="ptrans", bufs=2, space="PSUM") as ptrans_pool,
            tc.tile_pool(name="pcross", bufs=3, space="PSUM") as pcross_pool,
        ):
            # the first feature groups ride the Sync (HWDGE) queue,
            # which is ready before GpSimd's SWDGE preamble finishes;
            # issue them ahead of the consts so compute starts sooner
            fg_f32 = []
            o0 = 0
            for g in range(HWDGE_GROUPS):
                gs = GROUP_SIZES[g]
                fgf = featg_pool.tile([P, gs, D], f32, tag=f"fgf{g}")
                fv = feats[o0 * P : (o0 + gs) * P].rearrange(
                    "(p c) d -> p c d", p=P
                )
                nc.sync.dma_start(fgf[:], fv)
                fg_f32.append(fgf)
                o0 += gs

            id_s = consts.tile([P, P], bf16)
            nc.sync.dma_start(id_s[:], ident)
            ct_s = consts.tile([P, 2 * K], fp8)
            nc.sync.dma_start(ct_s[:], ctp)
            ct_ap = ct_s[:].rearrange("p (t k) -> p t k", t=2)
            c2m_s = consts.tile([P, 1], f32)
            nc.sync.dma_start(c2m_s[:], c2mean)

            # two manually alternated fT buffers, each holding TB chunk
            # slots of [2 ktiles x 128 cols] fp8.  Slice-1 partitions
            # 126/127 hold the constant 1.0 rows (c2a/c2b lhs side);
            # they are DMA-initialized once and never rewritten.
            ftbufs = []
            for b in range(FTBUFS):
                ft = consts.tile([P, TB * 2 * P], fp8, tag=f"ftbig{b}")
                ft4 = ft[:].rearrange("p (s t m) -> p s t m", s=TB, t=2)
                nc.sync.dma_start(ft4[P - 2 : P, :, 1, :], cones)
                ftbufs.append(ft4)

            m_coll = coll.tile([P, CHUNKS], f32)
            sqab_coll = coll.tile([P, 2 * CHUNKS], f32)
            sqab_v = sqab_coll[:].rearrange("p (i two) -> p i two", two=2)

            d2t = coll.tile([P, CHUNKS], f32)
            dist = coll.tile([P, CHUNKS], f32)
            sums = coll.tile([P, 2], f32)

            def tail_part(lo, hi, col):
                # sums[p, col] = sum_{i in [lo,hi)} sqrt(m + sq126 +
                # sq127 + c2mean); adds run on the mostly-idle Pool
                nc.gpsimd.tensor_add(
                    d2t[:, lo:hi], m_coll[:, lo:hi], sqab_v[:, lo:hi, 0]
                )
                nc.gpsimd.tensor_add(
                    d2t[:, lo:hi], d2t[:, lo:hi], sqab_v[:, lo:hi, 1]
                )
                nc.scalar.activation(
                    dist[:, lo:hi], d2t[:, lo:hi], Act.Sqrt,
                    bias=c2m_s[:], accum_out=sums[:, col : col + 1],
                )

            batch = 0
            o = 0
            for g, gs in enumerate(GROUP_SIZES):
                # group g covers chunks [o, o+gs); partition p takes gs
                # consecutive rows (contiguous descriptor per partition)
                fg = featg_pool.tile([P, gs, D], bf16, tag=f"fg{gs}")
                if g < HWDGE_GROUPS:
                    # already in SBUF as f32 (HWDGE); ACT casts to bf16
                    nc.scalar.copy(fg[:], fg_f32[g][:])
                else:
                    fview = feats[o * P : (o + gs) * P].rearrange(
                        "(p c) d -> p c d", p=P
                    )
                    nc.gpsimd.dma_start(fg[:], fview)  # SWDGE cast

                for h in range((gs + TB - 1) // TB):
                    bs = min(TB, gs - h * TB)
                    pt = ptrans_pool.tile([D, TB, P], bf16)
                    for j in range(bs):
                        c = h * TB + j
                        nc.tensor.transpose(pt[:, j, :], fg[:, c, :], id_s[:])
                    ft4 = ftbufs[batch % FTBUFS]
                    batch += 1
                    # evacuate: features (all 128 dims) and squares
                    # (dims 0..125) straight into the fp8 fT buffer
                    nc.scalar.copy(ft4[:, 0:bs, 0, :], pt[:, 0:bs, :])
                    nc.scalar.activation(
                        ft4[0 : P - 2, 0:bs, 1, :], pt[0 : P - 2, 0:bs, :],
                        Act.Square,
                    )

                    for cb in range(bs // TG):
                        px4 = pcross_pool.tile([P, TG, K], f32)
                        for j in range(TG):
                            s = cb * TG + j
                            nc.tensor.matmul(
                                px4[:, j, :],
                                ft4[:, s, :, :],
                                ct_ap,
                                start=True, stop=True,
                                perf_mode=mybir.MatmulPerfMode.DoubleRow,
                                skip_group_check=True,
                            )
                        ib = o + h * TB + cb * TG
                        nc.vector.tensor_reduce(
                            out=m_coll[:, ib : ib + TG],
                            in_=px4[:],
                            axis=mybir.AxisListType.X,
                            op=Alu.min,
                        )

                # dims 126/127 enter ||f||^2 via the tail instead of the
                # matmul (their ktile-1 rows carry the c2 constants)
                nc.gpsimd.tensor_mul(
                    sqab_v[:, o : o + gs, :],
                    fg[:, :, D - 2 : D],
                    fg[:, :, D - 2 : D],
                )
                o += gs
                if o == TAIL_SPLIT:
                    # overlap most of the tail (and its output DMA)
                    # with the last groups
                    tail_part(0, TAIL_SPLIT, 0)
                    nc.sync.dma_start(out[:, 0:1], sums[:, 0:1])

            tail_part(TAIL_SPLIT, CHUNKS, 1)
            nc.sync.dma_start(out[:, 1:2], sums[:, 1:2])

    nc.compile()
    return nc


def _get_compiled():
    global _compiled
    if _compiled is None:
        _compiled = _build()
    return _compiled


def _make_aux(centers: np.ndarray):
    import ml_dtypes

    e4 = ml_dtypes.float8_e4m3
    cen_bf = centers.astype(ml_dtypes.bfloat16).astype(np.float64)  # [K, D]
    ctneg2_8 = (-2.0 * cen_bf.T).astype(e4)                         # [D, K] fp8
    c_eff = -(ctneg2_8.astype(np.float64)) / 2.0                    # [D, K]
    c2 = (c_eff * c_eff).sum(axis=0)                                # [K]
    c2m = float(c2.mean())
    c2c = c2 - c2m
    c2a = c2c.astype(e4)
    c2b = (c2c - c2a.astype(np.float64)).astype(e4)

    # ctp[p, t*K + k]: t=0 -> -2c[k,p]; t=1 -> p<126: 1.0, p=126: c2a,
    # p=127: c2b
    ctp = np.zeros((P, 2 * K), dtype=e4)
    ctp[:, :K] = ctneg2_8
    ctp[: P - 2, K:] = e4(1.0)
    ctp[P - 2, K:] = c2a
    ctp[P - 1, K:] = c2b

    cones = np.full((2, TB * P), 1.0, dtype=e4)
    ident = np.eye(P, dtype=ml_dtypes.bfloat16)
    c2mean = np.full((P, 1), c2m, dtype=np.float32)
    return ctp, cones, ident, c2mean


def _make_in_maps(features: np.ndarray, centers: np.ndarray):
    ctp, cones, ident, c2mean = _make_aux(centers)
    return [
        {
            "features": features[c * N_PER_CORE : (c + 1) * N_PER_CORE],
            "ctp": ctp,
            "cones": cones,
            "ident": ident,
            "c2mean": c2mean,
        }
        for c in range(N_CORES)
    ]


def kernel(features: np.ndarray, centers: np.ndarray) -> np.ndarray:
    features = np.ascontiguousarray(np.asarray(features, dtype=np.float32))
    centers = np.ascontiguousarray(np.asarray(centers, dtype=np.float32))
    assert features.shape == (N_TOTAL, D) and centers.shape == (K, D)

    from concourse.bass_utils import run_bass_kernel_spmd

    nc = _get_compiled()
    in_maps = _make_in_maps(features, centers)
    res = run_bass_kernel_spmd(nc, in_maps, list(range(N_CORES)))
    total = 0.0
    for r in res.results:
        total += np.sum(r["out"].astype(np.float64))  # [P, 2] partial sums
    return np.float32(total / N_TOTAL)


if __name__ == "__main__":
    rng = np.random.default_rng(0)
    f = rng.standard_normal((N_TOTAL, D), dtype=np.float32)
    c = rng.standard_normal((K, D), dtype=np.float32)
    print(kernel(f, c))


# revision 41
# speedup vs baseline: 1.1105x; 1.1105x over previous
"""KMeans min-distance loss kernel for Trainium2 (8 NeuronCores, SPMD).

Problem: features [262144, 128] f32, centers [256, 128] f32.
  d2[n,k] = ||f_n||^2 + ||c_k||^2 - 2 f_n.c_k ; out = mean_n sqrt(min_k d2)

Sharding: data-parallel over N (32768 rows per core), centers replicated.
Each core returns [128] partial sums of min-distances; host reduces.

Per-core pipeline (fp8 DoubleRow matmul carries cross + f2 + c2):
  - SWDGE cast-DMA staged groups (0.5-2MB): f32 dram -> bf16 sbuf
  - PE transpose (bf16) chunks -> PSUM, batches of 8 per PSUM bank
  - ACT evacuates PSUM -> fT fp8: copy (features) + Square (squares)
  - one fp8 DoubleRow matmul per chunk, contraction 256:
      rows   0-127: f[n,d] x -2c[k,d]
      rows 128-253: f[n,d]^2 (d<126) x 1.0          (-> ||f||^2 partial)
      rows 254-255: 1.0 x c2a[k], 1.0 x c2b[k]      (-> centered ||c||^2,
                                                     fp8 error-feedback pair)
  - DVE segmented tensor_reduce min over k -> m[:, 4]  (the pacing
    engine: 64 x ~1.2us, PSUM f32 streams at 1 elem/cycle)
  - dims 126/127 squares on GpSimd from the [n,d] tile (their ktile-1
    rows carry the c2 constants instead)
  - tail in two parts (split overlaps the last group): Pool adds, ACT
    sqrt(d2 + mean_c2) accum -> out [128, 2] partial sums; host reduces
"""

import sys

for p in ("/opt/trn_rl_repo", "/opt/trn_rl_repo/concourse"):
    if p not in sys.path:
        sys.path.insert(0, p)

import numpy as np

N_TOTAL = 262144
K = 256
D = 128
N_CORES = 8
N_PER_CORE = N_TOTAL // N_CORES  # 32768
P = 128
CHUNKS = N_PER_CORE // P         # 256 chunks of 128 rows
TB = 8                           # chunks per transpose/evac batch
TG = 4                           # chunks per cross/reduce batch
FTBUFS = 4                       # fT buffer rotation depth
# staged DMA group sizes (chunks): small first groups so compute starts
# while the bulk still streams; 2MB groups at steady state; small final
# groups so the epilogue tail has little left to wait for
GROUP_SIZES = [4, 4, 8, 16] + [32] * 6 + [16] * 2
TAIL_SPLIT = 240                 # tail part A covers chunks [0, 240)

_compiled = None


def _build():
    import concourse.bass as bass
    import concourse.bacc as bacc
    import concourse.tile as tile
    from concourse import mybir

    f32 = mybir.dt.float32
    bf16 = mybir.dt.bfloat16
    fp8 = mybir.dt.float8e4
    Alu = mybir.AluOpType
    Act = mybir.ActivationFunctionType

    nc = bacc.Bacc(
        "TRN2", target_bir_lowering=False, debug=False, num_devices=N_CORES
    )

    feats = nc.dram_tensor("features", [N_PER_CORE, D], f32, kind="ExternalInput").ap()
    ctp = nc.dram_tensor("ctp", [P, 2 * K], fp8, kind="ExternalInput").ap()
    cones = nc.dram_tensor("cones", [2, TB * P], fp8, kind="ExternalInput").ap()
    ident = nc.dram_tensor("ident", [P, P], bf16, kind="ExternalInput").ap()
    c2mean = nc.dram_tensor("c2mean", [P, 1], f32, kind="ExternalInput").ap()
    out = nc.dram_tensor("out", [P, 2], f32, kind="ExternalOutput").ap()

    with tile.TileContext(nc) as tc:
        with (
            tc.tile_pool(name="consts", bufs=1) as consts,
            tc.tile_pool(name="featg", bufs=4) as featg_pool,
            tc.tile_pool(name="coll", bufs=1) as coll,
            tc.tile_pool(name="ptrans", bufs=2, space="PSUM") as ptrans_pool,
            tc.tile_pool(name="pcross", bufs=3, space="PSUM") as pcross_pool,
        ):
            id_s = consts.tile([P, P], bf16)
            nc.sync.dma_start(id_s[:], ident)
            ct_s = consts.tile([P, 2 * K], fp8)
            nc.sync.dma_start(ct_s[:], ctp)
            ct_ap = ct_s[:].rearrange("p (t k) -> p t k", t=2)
            c2m_s = consts.tile([P, 1], f32)
            nc.sync.dma_start(c2m_s[:], c2mean)

            # two manually alternated fT buffers, each holding TB chunk
            # slots of [2 ktiles x 128 cols] fp8.  Slice-1 partitions
            # 126/127 hold the constant 1.0 rows (c2a/c2b lhs side);
            # they are DMA-initialized once and never rewritten.
            ftbufs = []
            for b in range(FTBUFS):
                ft = consts.tile([P, TB * 2 * P], fp8, tag=f"ftbig{b}")
                ft4 = ft[:].rearrange("p (s t m) -> p s t m", s=TB, t=2)
                nc.sync.dma_start(ft4[P - 2 : P, :, 1, :], cones)
                ftbufs.append(ft4)

            m_coll = coll.tile([P, CHUNKS], f32)
            sqab_coll = coll.tile([P, 2 * CHUNKS], f32)
            sqab_v = sqab_coll[:].rearrange("p (i two) -> p i two", two=2)

            d2t = coll.tile([P, CHUNKS], f32)
            dist = coll.tile([P, CHUNKS], f32)
            sums = coll.tile([P, 2], f32)

            def tail_part(lo, hi, col):
                # sums[p, col] = sum_{i in [lo,hi)} sqrt(m + sq126 +
                # sq127 + c2mean); adds run on the mostly-idle Pool
                nc.gpsimd.tensor_add(
                    d2t[:, lo:hi], m_coll[:, lo:hi], sqab_v[:, lo:hi, 0]
                )
                nc.gpsimd.tensor_add(
                    d2t[:, lo:hi], d2t[:, lo:hi], sqab_v[:, lo:hi, 1]
                )
                nc.scalar.activation(
                    dist[:, lo:hi], d2t[:, lo:hi], Act.Sqrt,
                    bias=c2m_s[:], accum_out=sums[:, col : col + 1],
                )

            batch = 0
            o = 0
            for g, gs in enumerate(GROUP_SIZES):
                # group g covers chunks [o, o+gs); partition p takes gs
                # consecutive rows (contiguous descriptor per partition)
                fg = featg_pool.tile([P, gs, D], bf16, tag=f"fg{gs}")
                fview = feats[o * P : (o + gs) * P].rearrange(
                    "(p c) d -> p c d", p=P
                )
                nc.gpsimd.dma_start(fg[:], fview)  # SWDGE cast f32->bf16

                for h in range((gs + TB - 1) // TB):
                    bs = min(TB, gs - h * TB)
                    pt = ptrans_pool.tile([D, TB, P], bf16)
                    for j in range(bs):
                        c = h * TB + j
                        nc.tensor.transpose(pt[:, j, :], fg[:, c, :], id_s[:])
                    ft4 = ftbufs[batch % FTBUFS]
                    batch += 1
                    # evacuate: features (all 128 dims) and squares
                    # (dims 0..125) straight into the fp8 fT buffer
                    nc.scalar.copy(ft4[:, 0:bs, 0, :], pt[:, 0:bs, :])
                    nc.scalar.activation(
                        ft4[0 : P - 2, 0:bs, 1, :], pt[0 : P - 2, 0:bs, :],
                        Act.Square,
                    )

                    for cb in range(bs // TG):
                        px4 = pcross_pool.tile([P, TG, K], f32)
                        for j in range(TG):
                            s = cb * TG + j
                            nc.tensor.matmul(
                                px4[:, j, :],
                                ft4[:, s, :, :],
                                ct_ap,
                                start=True, stop=True,
                                perf_mode=mybir.MatmulPerfMode.DoubleRow,
                                skip_group_check=True,
                            )
                        ib = o + h * TB + cb * TG
                        nc.vector.tensor_reduce(
                            out=m_coll[:, ib : ib + TG],
                            in_=px4[:],
                            axis=mybir.AxisListType.X,
                            op=Alu.min,
                        )

                # dims 126/127 enter ||f||^2 via the tail instead of the
                # matmul (their ktile-1 rows carry the c2 constants)
                nc.gpsimd.tensor_mul(
                    sqab_v[:, o : o + gs, :],
                    fg[:, :, D - 2 : D],
                    fg[:, :, D - 2 : D],
                )
                o += gs
                if o == TAIL_SPLIT:
                    # overlap most of the tail (and its output DMA)
                    # with the last groups
                    tail_part(0, TAIL_SPLIT, 0)
                    nc.sync.dma_start(out[:, 0:1], sums[:, 0:1])

            tail_part(TAIL_SPLIT, CHUNKS, 1)
            nc.sync.dma_start(out[:, 1:2], sums[:, 1:2])

    nc.compile()
    return nc


def _get_compiled():
    global _compiled
    if _compiled is None:
        _compiled = _build()
    return _compiled


def _make_aux(centers: np.ndarray):
    import ml_dtypes

    e4 = ml_dtypes.float8_e4m3
    cen_bf = centers.astype(ml_dtypes.bfloat16).astype(np.float64)  # [K, D]
    ctneg2_8 = (-2.0 * cen_bf.T).astype(e4)                         # [D, K] fp8
    c_eff = -(ctneg2_8.astype(np.float64)) / 2.0                    # [D, K]
    c2 = (c_eff * c_eff).sum(axis=0)                                # [K]
    c2m = float(c2.mean())
    c2c = c2 - c2m
    c2a = c2c.astype(e4)
    c2b = (c2c - c2a.astype(np.float64)).astype(e4)

    # ctp[p, t*K + k]: t=0 -> -2c[k,p]; t=1 -> p<126: 1.0, p=126: c2a,
    # p=127: c2b
    ctp = np.zeros((P, 2 * K), dtype=e4)
    ctp[:, :K] = ctneg2_8
    ctp[: P - 2, K:] = e4(1.0)
    ctp[P - 2, K:] = c2a
    ctp[P - 1, K:] = c2b

    cones = np.full((2, TB * P), 1.0, dtype=e4)
    ident = np.eye(P, dtype=ml_dtypes.bfloat16)
    c2mean = np.full((P, 1), c2m, dtype=np.float32)
    return ctp, cones, ident, c2mean


def _make_in_maps(features: np.ndarray, centers: np.ndarray):
    ctp, cones, ident, c2mean = _make_aux(centers)
    return [
        {
            "features": features[c * N_PER_CORE : (c + 1) * N_PER_CORE],
            "ctp": ctp,
            "cones": cones,
            "ident": ident,
            "c2mean": c2mean,
        }
        for c in range(N_CORES)
    ]


def kernel(features: np.ndarray, centers: np.ndarray) -> np.ndarray:
    features = np.ascontiguousarray(np.asarray(features, dtype=np.float32))
    centers = np.ascontiguousarray(np.asarray(centers, dtype=np.float32))
    assert features.shape == (N_TOTAL, D) and centers.shape == (K, D)

    from concourse.bass_utils import run_bass_kernel_spmd

    nc = _get_compiled()
    in_maps = _make_in_maps(features, centers)
    res = run_bass_kernel_spmd(nc, in_maps, list(range(N_CORES)))
    total = 0.0
    for r in res.results:
        total += np.sum(r["out"].astype(np.float64))  # [P, 2] partial sums
    return np.float32(total / N_TOTAL)


if __name__ == "__main__":
    rng = np.random.default_rng(0)
    f = rng.standard_normal((N_TOTAL, D), dtype=np.float32)
    c = rng.standard_normal((K, D), dtype=np.float32)
    print(kernel(f, c))


# revision 42
# speedup vs baseline: 1.1260x; 1.0140x over previous
"""KMeans min-distance loss kernel for Trainium2 (8 NeuronCores, SPMD).

Problem: features [262144, 128] f32, centers [256, 128] f32.
  d2[n,k] = ||f_n||^2 + ||c_k||^2 - 2 f_n.c_k ; out = mean_n sqrt(min_k d2)

Sharding: data-parallel over N (32768 rows per core), centers replicated.
Each core returns [128] partial sums of min-distances; host reduces.

Per-core pipeline (fp8 DoubleRow matmul carries cross + f2 + c2):
  - SWDGE cast-DMA staged groups (0.5-2MB): f32 dram -> bf16 sbuf
  - PE transpose (bf16) chunks -> PSUM, batches of 8 per PSUM bank
  - ACT evacuates PSUM -> fT fp8: copy (features) + Square (squares)
  - one fp8 DoubleRow matmul per chunk, contraction 256:
      rows   0-127: f[n,d] x -2c[k,d]
      rows 128-253: f[n,d]^2 (d<126) x 1.0          (-> ||f||^2 partial)
      rows 254-255: 1.0 x c2a[k], 1.0 x c2b[k]      (-> centered ||c||^2,
                                                     fp8 error-feedback pair)
  - DVE segmented tensor_reduce min over k -> m[:, 4]  (the pacing
    engine: 64 x ~1.2us, PSUM f32 streams at 1 elem/cycle)
  - dims 126/127 squares on GpSimd from the [n,d] tile (their ktile-1
    rows carry the c2 constants instead)
  - tail in two parts (split overlaps the last group): Pool adds, ACT
    sqrt(d2 + mean_c2) accum -> out [128, 2] partial sums; host reduces
"""

import sys

for p in ("/opt/trn_rl_repo", "/opt/trn_rl_repo/concourse"):
    if p not in sys.path:
        sys.path.insert(0, p)

import numpy as np

N_TOTAL = 262144
K = 256
D = 128
N_CORES = 8
N_PER_CORE = N_TOTAL // N_CORES  # 32768
P = 128
CHUNKS = N_PER_CORE // P         # 256 chunks of 128 rows
TB = 8                           # chunks per transpose/evac batch
TG = 4                           # chunks per cross/reduce batch
FTBUFS = 4                       # fT buffer rotation depth
# staged DMA group sizes (chunks): small first groups so compute starts
# while the bulk still streams; 2MB groups at steady state; small final
# groups so the epilogue tail has little left to wait for
GROUP_SIZES = [4, 4, 8, 16, 32, 64, 64, 32, 16, 16]
TAIL_SPLIT = 240                 # tail part A covers chunks [0, 240)

_compiled = None


def _build():
    import concourse.bass as bass
    import concourse.bacc as bacc
    import concourse.tile as tile
    from concourse import mybir

    f32 = mybir.dt.float32
    bf16 = mybir.dt.bfloat16
    fp8 = mybir.dt.float8e4
    Alu = mybir.AluOpType
    Act = mybir.ActivationFunctionType

    nc = bacc.Bacc(
        "TRN2", target_bir_lowering=False, debug=False, num_devices=N_CORES
    )

    feats = nc.dram_tensor("features", [N_PER_CORE, D], f32, kind="ExternalInput").ap()
    ctp = nc.dram_tensor("ctp", [P, 2 * K], fp8, kind="ExternalInput").ap()
    cones = nc.dram_tensor("cones", [2, TB * P], fp8, kind="ExternalInput").ap()
    ident = nc.dram_tensor("ident", [P, P], bf16, kind="ExternalInput").ap()
    c2mean = nc.dram_tensor("c2mean", [P, 1], f32, kind="ExternalInput").ap()
    out = nc.dram_tensor("out", [P, 2], f32, kind="ExternalOutput").ap()

    with tile.TileContext(nc) as tc:
        with (
            tc.tile_pool(name="consts", bufs=1) as consts,
            tc.tile_pool(name="featg", bufs=4) as featg_pool,
            tc.tile_pool(name="coll", bufs=1) as coll,
            tc.tile_pool(name="ptrans", bufs=2, space="PSUM") as ptrans_pool,
            tc.tile_pool(name="pcross", bufs=3, space="PSUM") as pcross_pool,
        ):
            id_s = consts.tile([P, P], bf16)
            nc.sync.dma_start(id_s[:], ident)
            ct_s = consts.tile([P, 2 * K], fp8)
            nc.sync.dma_start(ct_s[:], ctp)
            ct_ap = ct_s[:].rearrange("p (t k) -> p t k", t=2)
            c2m_s = consts.tile([P, 1], f32)
            nc.sync.dma_start(c2m_s[:], c2mean)

            # two manually alternated fT buffers, each holding TB chunk
            # slots of [2 ktiles x 128 cols] fp8.  Slice-1 partitions
            # 126/127 hold the constant 1.0 rows (c2a/c2b lhs side);
            # they are DMA-initialized once and never rewritten.
            ftbufs = []
            for b in range(FTBUFS):
                ft = consts.tile([P, TB * 2 * P], fp8, tag=f"ftbig{b}")
                ft4 = ft[:].rearrange("p (s t m) -> p s t m", s=TB, t=2)
                nc.sync.dma_start(ft4[P - 2 : P, :, 1, :], cones)
                ftbufs.append(ft4)

            m_coll = coll.tile([P, CHUNKS], f32)
            sqab_coll = coll.tile([P, 2 * CHUNKS], f32)
            sqab_v = sqab_coll[:].rearrange("p (i two) -> p i two", two=2)

            d2t = coll.tile([P, CHUNKS], f32)
            dist = coll.tile([P, CHUNKS], f32)
            sums = coll.tile([P, 2], f32)

            def tail_part(lo, hi, col):
                # sums[p, col] = sum_{i in [lo,hi)} sqrt(m + sq126 +
                # sq127 + c2mean); adds run on the mostly-idle Pool
                nc.gpsimd.tensor_add(
                    d2t[:, lo:hi], m_coll[:, lo:hi], sqab_v[:, lo:hi, 0]
                )
                nc.gpsimd.tensor_add(
                    d2t[:, lo:hi], d2t[:, lo:hi], sqab_v[:, lo:hi, 1]
                )
                nc.scalar.activation(
                    dist[:, lo:hi], d2t[:, lo:hi], Act.Sqrt,
                    bias=c2m_s[:], accum_out=sums[:, col : col + 1],
                )

            batch = 0
            o = 0
            for g, gs in enumerate(GROUP_SIZES):
                # group g covers chunks [o, o+gs); partition p takes gs
                # consecutive rows (contiguous descriptor per partition)
                fg = featg_pool.tile([P, gs, D], bf16, tag=f"fg{gs}")
                fview = feats[o * P : (o + gs) * P].rearrange(
                    "(p c) d -> p c d", p=P
                )
                nc.gpsimd.dma_start(fg[:], fview)  # SWDGE cast f32->bf16

                for h in range((gs + TB - 1) // TB):
                    bs = min(TB, gs - h * TB)
                    pt = ptrans_pool.tile([D, TB, P], bf16)
                    for j in range(bs):
                        c = h * TB + j
                        nc.tensor.transpose(pt[:, j, :], fg[:, c, :], id_s[:])
                    ft4 = ftbufs[batch % FTBUFS]
                    batch += 1
                    # evacuate: features (all 128 dims) and squares
                    # (dims 0..125) straight into the fp8 fT buffer
                    nc.scalar.copy(ft4[:, 0:bs, 0, :], pt[:, 0:bs, :])
                    nc.scalar.activation(
                        ft4[0 : P - 2, 0:bs, 1, :], pt[0 : P - 2, 0:bs, :],
                        Act.Square,
                    )

                    for cb in range(bs // TG):
                        px4 = pcross_pool.tile([P, TG, K], f32)
                        for j in range(TG):
                            s = cb * TG + j
                            nc.tensor.matmul(
                                px4[:, j, :],
                                ft4[:, s, :, :],
                                ct_ap,
                                start=True, stop=True,
                                perf_mode=mybir.MatmulPerfMode.DoubleRow,
                                skip_group_check=True,
                            )
                        ib = o + h * TB + cb * TG
                        nc.vector.tensor_reduce(
                            out=m_coll[:, ib : ib + TG],
                            in_=px4[:],
                            axis=mybir.AxisListType.X,
                            op=Alu.min,
                        )

                # dims 126/127 enter ||f||^2 via the tail instead of the
                # matmul (their ktile-1 rows carry the c2 constants)
                nc.gpsimd.tensor_mul(
                    sqab_v[:, o : o + gs, :],
                    fg[:, :, D - 2 : D],
                    fg[:, :, D - 2 : D],
                )
                o += gs
                if o == TAIL_SPLIT:
                    # overlap most of the tail (and its output DMA)
                    # with the last groups
                    tail_part(0, TAIL_SPLIT, 0)
                    nc.sync.dma_start(out[:, 0:1], sums[:, 0:1])

            tail_part(TAIL_SPLIT, CHUNKS, 1)
            nc.sync.dma_start(out[:, 1:2], sums[:, 1:2])

    nc.compile()
    return nc


def _get_compiled():
    global _compiled
    if _compiled is None:
        _compiled = _build()
    return _compiled


def _make_aux(centers: np.ndarray):
    import ml_dtypes

    e4 = ml_dtypes.float8_e4m3
    cen_bf = centers.astype(ml_dtypes.bfloat16).astype(np.float64)  # [K, D]
    ctneg2_8 = (-2.0 * cen_bf.T).astype(e4)                         # [D, K] fp8
    c_eff = -(ctneg2_8.astype(np.float64)) / 2.0                    # [D, K]
    c2 = (c_eff * c_eff).sum(axis=0)                                # [K]
    c2m = float(c2.mean())
    c2c = c2 - c2m
    c2a = c2c.astype(e4)
    c2b = (c2c - c2a.astype(np.float64)).astype(e4)

    # ctp[p, t*K + k]: t=0 -> -2c[k,p]; t=1 -> p<126: 1.0, p=126: c2a,
    # p=127: c2b
    ctp = np.zeros((P, 2 * K), dtype=e4)
    ctp[:, :K] = ctneg2_8
    ctp[: P - 2, K:] = e4(1.0)
    ctp[P - 2, K:] = c2a
    ctp[P - 1, K:] = c2b

    cones = np.full((2, TB * P), 1.0, dtype=e4)
    ident = np.eye(P, dtype=ml_dtypes.bfloat16)
    c2mean = np.full((P, 1), c2m, dtype=np.float32)
    return ctp, cones, ident, c2mean


def _make_in_maps(features: np.ndarray, centers: np.ndarray):
    ctp, cones, ident, c2mean = _make_aux(centers)
    return [
        {
            "features": features[c * N_PER_CORE : (c + 1) * N_PER_CORE],
            "ctp": ctp,
            "cones": cones,
            "ident": ident,
            "c2mean": c2mean,
        }
        for c in range(N_CORES)
    ]


def kernel(features: np.ndarray, centers: np.ndarray) -> np.ndarray:
    features = np.ascontiguousarray(np.asarray(features, dtype=np.float32))
    centers = np.ascontiguousarray(np.asarray(centers, dtype=np.float32))
    assert features.shape == (N_TOTAL, D) and centers.shape == (K, D)

    from concourse.bass_utils import run_bass_kernel_spmd

    nc = _get_compiled()
    in_maps = _make_in_maps(features, centers)
    res = run_bass_kernel_spmd(nc, in_maps, list(range(N_CORES)))
    total = 0.0
    for r in res.results:
        total += np.sum(r["out"].astype(np.float64))  # [P, 2] partial sums
    return np.float32(total / N_TOTAL)


if __name__ == "__main__":
    rng = np.random.default_rng(0)
    f = rng.standard_normal((N_TOTAL, D), dtype=np.float32)
    c = rng.standard_normal((K, D), dtype=np.float32)
    print(kernel(f, c))
